# revision 1
# baseline (speedup 1.0000x reference)
"""Trainium2 Bass kernel for a GPT-style transformer block.

Problem: nn_Block_36807869727037 (dense_transformer)
  B=4, T=2048, C=1024, H=16 heads (d=64), fp32 I/O.
  y = x + attn(LN1(x)); y = y + mlp(LN2(y))  (causal attention, tanh-GELU MLP)

Sharding (8 cores, one uniform SPMD program):
  core = 2*b + s  -> batch b in [0,4), tensor-parallel shard s in [0,2).
  Shard s owns heads [8s, 8s+8) and FFN hidden slice [2048s, 2048s+2048).
  Each core runs the full sequence (T=2048) for its batch:
    LN1 (duplicated in pair) -> QKV for its 8 heads -> causal attention ->
    c_proj partial -> pairwise AllReduce (bf16) -> x2 = x + attn + b_proj ->
    LN2 (duplicated) -> fc half + GELU -> mlp_proj partial.
  Final combine on host: out[b] = x2 (from core 2b) + mlp_partial(2b)
                                  + mlp_partial(2b+1) + b_mlp_proj.

Layouts: activations enter matmuls feature-major ("X^T", features on
partitions) as rhs; weights host-pre-transposed/sliced, bf16. Scores are
computed transposed S^T[s,q] so softmax needs no max-subtraction
(|scores/8| < ~3) and the denominator comes from an appended ones-column
in V. All matmul operands bf16, PSUM/LN/residuals fp32.
"""

import os
import sys

sys.path.insert(0, "/opt/trn_rl_repo")

import numpy as np
import ml_dtypes

B, T, C, H = 4, 2048, 1024, 16
D = C // H          # 64 head dim
HPC = H // 2        # 8 heads per core
FPC = 2 * C         # 2048 ffn hidden per core
P = 128
EPS = 1e-10
NT = T // P         # 16 token tiles
NR = T // 512       # 4 query runs of 512
CK = C // P         # 8 feature chunks
FK = FPC // P       # 16 ffn chunks per core
EK = (HPC * D) // P  # 4 head-dim chunks per core (512/128)

_CACHED = {}


def _build_nc():
    import concourse.bass as bass
    import concourse.mybir as mybir
    import concourse.tile as tile
    from concourse import bacc

    f32 = mybir.dt.float32
    bf16 = mybir.dt.bfloat16
    AF = mybir.ActivationFunctionType
    ALU = mybir.AluOpType

    nc = bacc.Bacc(trn_type="TRN2", target_bir_lowering=False, num_devices=8)

    # ---- I/O ----
    x_d = nc.dram_tensor("x", [T, C], f32, kind="ExternalInput")
    wqT_d = nc.dram_tensor("wqT", [C, HPC * D], bf16, kind="ExternalInput")
    wkT_d = nc.dram_tensor("wkT", [C, HPC * D], bf16, kind="ExternalInput")
    wvT_d = nc.dram_tensor("wvT", [C, HPC * D], bf16, kind="ExternalInput")
    wpT_d = nc.dram_tensor("wpT", [HPC * D, C], bf16, kind="ExternalInput")
    wfT_d = nc.dram_tensor("wfT", [C, FPC], bf16, kind="ExternalInput")
    wmT_d = nc.dram_tensor("wmT", [FPC, C], bf16, kind="ExternalInput")
    bqk_d = nc.dram_tensor("bqk", [P, 2 * EK], f32, kind="ExternalInput")
    bv_d = nc.dram_tensor("bv", [HPC * D], f32, kind="ExternalInput")
    bproj_d = nc.dram_tensor("bproj", [C], f32, kind="ExternalInput")
    bfc_d = nc.dram_tensor("bfc", [P, FK], f32, kind="ExternalInput")
    mask_d = nc.dram_tensor("mask", [P, P], bf16, kind="ExternalInput")

    out_mlp_d = nc.dram_tensor("out_mlp", [T, C], f32, kind="ExternalOutput")
    out_x2_d = nc.dram_tensor("out_x2", [T, C], f32, kind="ExternalOutput")

    cc_in_d = nc.dram_tensor("cc_in", [T, C], bf16)
    cc_out_d = nc.dram_tensor("cc_out", [T, C], bf16)

    def bcast_row(dram_ap, n):
        # [n] DRAM vector -> [P, n] broadcast AP (partition-step 0)
        return bass.AP(
            tensor=dram_ap.tensor, offset=dram_ap.offset,
            ap=[[0, P], *dram_ap.ap],
        )

    with tile.TileContext(nc, pool_alloc_mode="queue") as tc:
        import contextlib

        with contextlib.ExitStack() as ctx:
            consts = ctx.enter_context(tc.tile_pool(name="consts", bufs=1))
            work = ctx.enter_context(tc.tile_pool(name="work", bufs=3))
            ln_pool = ctx.enter_context(tc.tile_pool(name="ln", bufs=4))
            small = ctx.enter_context(tc.tile_pool(name="small", bufs=2))
            ppool = ctx.enter_context(tc.tile_pool(name="psum", bufs=2, space="PSUM"))
            scpool = ctx.enter_context(
                tc.tile_pool(name="psum_sc", bufs=2, space="PSUM"))
            pvpool = ctx.enter_context(
                tc.tile_pool(name="psum_pv", bufs=1, space="PSUM"))
            bcpool = ctx.enter_context(
                tc.tile_pool(name="psum_bc", bufs=1, space="PSUM"))

            # ---- constants ----
            mask_sb = consts.tile([P, P], bf16)
            nc.scalar.dma_start(mask_sb[:], mask_d[:])
            bqk_sb = consts.tile([P, 2 * EK], f32)
            nc.scalar.dma_start(bqk_sb[:], bqk_d[:])
            bfc_sb = consts.tile([P, FK], f32)
            nc.scalar.dma_start(bfc_sb[:], bfc_d[:])
            bv_sb = consts.tile([P, HPC * D], f32)
            nc.scalar.dma_start(bv_sb[:], bcast_row(bv_d[:], HPC * D))
            bproj_sb = consts.tile([P, C], f32)
            nc.scalar.dma_start(bproj_sb[:], bcast_row(bproj_d[:], C))
            eps_sb = consts.tile([P, 1], f32)
            nc.vector.memset(eps_sb[:], EPS)
            ones_sb = consts.tile([1, D], bf16)
            nc.vector.memset(ones_sb[:], 1.0)
            from concourse.masks import make_identity
            ident_sb = consts.tile([P, P], bf16)
            make_identity(nc, ident_sb)

            def layernorm_tile(x_sb, out_bf):
                # x_sb [P, C] fp32 -> out_bf [P, C] bf16 normalized
                xg = x_sb[:].rearrange("p (g f) -> p g f", f=512)
                stats = ln_pool.tile([P, 2, 6], f32, tag="ln_stats")
                for g in range(2):
                    nc.vector.bn_stats(out=stats[:, g, :], in_=xg[:, g, :])
                mv = ln_pool.tile([P, 2], f32, tag="ln_mv")
                nc.vector.bn_aggr(out=mv[:], in_=stats[:])
                std = ln_pool.tile([P, 1], f32, tag="ln_std")
                nc.scalar.activation(
                    out=std[:], in_=mv[:, 1:2], func=AF.Sqrt,
                    bias=eps_sb[:], scale=1.0,
                )
                nc.vector.reciprocal(out=std[:], in_=std[:])
                nc.vector.tensor_scalar(
                    out=out_bf[:], in0=x_sb[:],
                    scalar1=mv[:, 0:1], scalar2=std[:],
                    op0=ALU.subtract, op1=ALU.mult,
                )

            # persistent activation tensors (released before MLP)
            attn_cm = tc.tile_pool(name="attn", bufs=1)
            attn_pool = attn_cm.__enter__()
            QT = attn_pool.tile([P, EK, T], bf16)
            KT = attn_pool.tile([P, EK, T], bf16)
            V_aug = attn_pool.tile([P, NT, HPC, D + 1], bf16)
            OT = attn_pool.tile([P, EK, T], bf16)
            nc.vector.memset(V_aug[:, :, :, D : D + 1], 1.0)

            xnT_cm = tc.tile_pool(name="p_xnT", bufs=1)
            p_xnT = xnT_cm.__enter__()
            xnT = p_xnT.tile([P, CK, T], bf16)

            wearly_cm = tc.tile_pool(name="wearly", bufs=1)
            wearly = wearly_cm.__enter__()
            wq_sb = wearly.tile([P, CK, HPC * D], bf16)
            wk_sb = wearly.tile([P, CK, HPC * D], bf16)
            wv_sb = wearly.tile([P, CK, HPC * D], bf16)

            def emit_wearly_dmas():
                nc.gpsimd.dma_start(
                    wv_sb[:], wvT_d.ap().rearrange("(k p) o -> p k o", p=P))
                nc.scalar.dma_start(
                    wq_sb[:], wqT_d.ap().rearrange("(k p) o -> p k o", p=P))
                nc.scalar.dma_start(
                    wk_sb[:], wkT_d.ap().rearrange("(k p) o -> p k o", p=P))

            wp_cm = tc.tile_pool(name="wp", bufs=1)
            wp_pool = wp_cm.__enter__()
            wp_sb = wp_pool.tile([P, EK, C], bf16)
            nc.scalar.dma_start(
                wp_sb[:], wpT_d.ap().rearrange("(k p) o -> p k o", p=P))

            pt_cm = tc.tile_pool(name="ptp", bufs=4)
            pt_pool = pt_cm.__enter__()

            xn2T_cm = tc.tile_pool(name="p_xn2T", bufs=1, side="right")
            p_xn2T = xn2T_cm.__enter__()
            xn2T = p_xn2T.tile([P, CK, T], bf16)

            def emit_ln1(tt):
                x_sb = work.tile([P, C], f32, tag="f32buf")
                nc.gpsimd.dma_start(x_sb[:], x_d[tt * P : (tt + 1) * P, :])
                xn_bf = work.tile([P, C], bf16, tag="bf16buf")
                layernorm_tile(x_sb, xn_bf)
                nc.sync.dma_start_transpose(
                    xnT[:, :, tt * P : (tt + 1) * P], xn_bf[:])

            def emit_x2_run(rr):
                # x2 = x + attn + b_proj; LN2; transpose (for run rr)
                for tt in range(4 * rr, 4 * rr + 4):
                    x_sb = work.tile([P, C], f32, tag="f32buf")
                    nc.gpsimd.dma_start(x_sb[:], x_d[tt * P : (tt + 1) * P, :])
                    att_sb = work.tile([P, C], bf16, tag="bf16buf")
                    nc.gpsimd.dma_start(
                        att_sb[:], cc_out_d[tt * P : (tt + 1) * P, :])
                    x2_sb = work.tile([P, C], f32, tag="f32buf")
                    nc.vector.tensor_add(
                        out=x2_sb[:], in0=x_sb[:], in1=att_sb[:])
                    nc.vector.tensor_add(
                        out=x2_sb[:], in0=x2_sb[:], in1=bproj_sb[:])
                    nc.gpsimd.dma_start(
                        out_x2_d[tt * P : (tt + 1) * P, :], x2_sb[:])
                    xn2_bf = work.tile([P, C], bf16, tag="bf16buf")
                    layernorm_tile(x2_sb, xn2_bf)
                    trp = scpool.tile([P, 1024], f32, tag="sc")
                    trpb = trp[:].bitcast(bf16)
                    for ck in range(CK):
                        nc.tensor.transpose(
                            trpb[:, ck * P : (ck + 1) * P],
                            xn2_bf[:, ck * P : (ck + 1) * P],
                            ident_sb[:],
                        )
                    nc.vector.tensor_copy(
                        out=xn2T[:, :, tt * P : (tt + 1) * P],
                        in_=trpb[:, : C].rearrange("p (c t) -> p c t", c=CK),
                    )

            # ======== fused pipeline over the 4 token runs ========
            for r in range(NR):
                # LN1 for the NEXT run's tiles (this run's was emitted earlier)
                if r == 0:
                    for tt in range(0, 4):
                        emit_ln1(tt)
                    emit_wearly_dmas()
                if r < NR - 1:
                    for tt in range(4 * (r + 1), 4 * (r + 1) + 4):
                        emit_ln1(tt)
                # --- V matmuls for this run's 4 token tiles ---
                for tt in range(4 * r, 4 * r + 4):
                    ps = ppool.tile([P, 512], f32, tag="mm")
                    for ck in range(CK):
                        nc.tensor.matmul(
                            ps[:],
                            xnT[:, ck, tt * P : (tt + 1) * P],
                            wv_sb[:, ck, :],
                            start=(ck == 0), stop=(ck == CK - 1),
                        )
                    nc.vector.tensor_add(
                        out=V_aug[:, tt, :, 0:D],
                        in0=ps[:].rearrange("p (h e) -> p h e", h=HPC),
                        in1=bv_sb[:].rearrange("p (h e) -> p h e", h=HPC),
                    )

                # --- Q^T, K^T for this run ---
                for ot in range(2 * EK):  # 0-3 Q tiles, 4-7 K tiles
                    w_sb = wq_sb if ot < EK else wk_sb
                    ol = (ot % EK) * P
                    dst = QT if ot < EK else KT
                    ps = ppool.tile([P, 512], f32, tag="mm")
                    for ck in range(CK):
                        nc.tensor.matmul(
                            ps[:],
                            w_sb[:, ck, ol : ol + P],
                            xnT[:, ck, r * 512 : (r + 1) * 512],
                            start=(ck == 0), stop=(ck == CK - 1),
                        )
                    nc.vector.tensor_scalar_add(
                        out=dst[:, ot % EK, r * 512 : (r + 1) * 512],
                        in0=ps[:], scalar1=bqk_sb[:, ot : ot + 1],
                    )

                # --- attention for this query run, all 8 heads ---
                ns = 4 * r + 4
                npairs = ns // 2
                for h in range(HPC):
                    hp = (h % 2) * D
                    hc = h // 2
                    po = pvpool.tile([P, 512], f32, tag="pv")

                    def emit_spair(pi):
                        sc = scpool.tile([P, 1024], f32, tag="sc")
                        for half in range(2):
                            st = 2 * pi + half
                            nc.tensor.matmul(
                                sc[:, half * 512 : (half + 1) * 512],
                                KT[hp : hp + D, hc, st * P : (st + 1) * P],
                                QT[hp : hp + D, hc, r * 512 : (r + 1) * 512],
                                start=True, stop=True,
                            )
                        return sc

                    def emit_exp_pv(pi, sc):
                        PT = pt_pool.tile([P, 1024], bf16, tag="PT")
                        lo = 2 * pi - 4 * r  # j of first half
                        if lo < -1:
                            # both halves fully below diagonal
                            nc.scalar.activation(
                                out=PT[:], in_=sc[:], func=AF.Exp, scale=0.125)
                        else:
                            for half in range(2):
                                st = 2 * pi + half
                                j = st - 4 * r
                                off = half * 512
                                if j < 0:
                                    nc.scalar.activation(
                                        out=PT[:, off : off + 512],
                                        in_=sc[:, off : off + 512],
                                        func=AF.Exp, scale=0.125)
                                else:
                                    nc.scalar.activation(
                                        out=PT[:, off + j * P : off + 512],
                                        in_=sc[:, off + j * P : off + 512],
                                        func=AF.Exp, scale=0.125)
                                    nc.vector.tensor_mul(
                                        out=PT[:, off + j * P : off + (j + 1) * P],
                                        in0=PT[:, off + j * P : off + (j + 1) * P],
                                        in1=mask_sb[:],
                                    )
                                    if j > 0:
                                        nc.vector.memset(
                                            PT[:, off : off + j * P], 0.0)
                        for half in range(2):
                            st = 2 * pi + half
                            nc.tensor.matmul(
                                po[: D + 1, :],
                                V_aug[:, st, h, 0 : D + 1],
                                PT[:, half * 512 : (half + 1) * 512],
                                start=(st == 0), stop=(st == ns - 1),
                            )

                    sc_prev = emit_spair(0)
                    for pi in range(npairs):
                        sc_next = emit_spair(pi + 1) if pi + 1 < npairs else None
                        emit_exp_pv(pi, sc_prev)
                        sc_prev = sc_next

                    if h == 5 and r >= 1:
                        emit_x2_run(r - 1)
                    dsum = small.tile([1, 512], f32, tag="dsum")
                    nc.vector.tensor_copy(out=dsum[:], in_=po[D : D + 1, :])
                    rec = small.tile([1, 512], f32, tag="rec")
                    nc.vector.reciprocal_approx_fast(out=rec[:], in_=dsum[:])
                    rec_bf = small.tile([1, 512], bf16, tag="recbf")
                    nc.vector.tensor_copy(out=rec_bf[:], in_=rec[:])
                    pb = bcpool.tile([D, 512], f32, tag="bcast")
                    nc.tensor.matmul(
                        pb[:], ones_sb[:], rec_bf[:], start=True, stop=True)
                    den_sb = small.tile([D, 512], f32, tag="den")
                    nc.vector.tensor_copy(out=den_sb[:], in_=pb[:])
                    nc.vector.tensor_mul(
                        out=OT[hp : hp + D, hc, r * 512 : (r + 1) * 512],
                        in0=po[0:D, :],
                        in1=den_sb[:],
                    )

                # --- c_proj partial for this run's tiles + AllReduce chunk ---
                for tt in range(4 * r, 4 * r + 4):
                    cc_sb = work.tile([P, C], bf16, tag="bf16buf")
                    for half in range(2):
                        ps = ppool.tile([P, 512], f32, tag="mm")
                        for ek in range(EK):
                            nc.tensor.matmul(
                                ps[:],
                                OT[:, ek, tt * P : (tt + 1) * P],
                                wp_sb[:, ek, half * 512 : (half + 1) * 512],
                                start=(ek == 0), stop=(ek == EK - 1),
                            )
                        nc.vector.tensor_copy(
                            out=cc_sb[:, half * 512 : (half + 1) * 512],
                            in_=ps[:])
                    nc.gpsimd.dma_start(
                        cc_in_d[tt * P : (tt + 1) * P, :], cc_sb[:])

                nc.gpsimd.collective_compute(
                    "AllReduce",
                    ALU.add,
                    replica_groups=[[0, 1], [2, 3], [4, 5], [6, 7]],
                    ins=[cc_in_d[r * 512 : (r + 1) * 512, :].opt()],
                    outs=[cc_out_d[r * 512 : (r + 1) * 512, :].opt()],
                )


            # release attention-phase SBUF before the MLP phase
            pt_cm.__exit__(None, None, None)
            wp_cm.__exit__(None, None, None)
            wearly_cm.__exit__(None, None, None)
            xnT_cm.__exit__(None, None, None)
            attn_cm.__exit__(None, None, None)

            with tc.tile_pool(name="wlate", bufs=1, side="right") as wlate, \
                 tc.tile_pool(name="p_hT", bufs=2, side="right") as p_hT:
                wf_sb = wlate.tile([P, CK, FPC], bf16)
                nc.scalar.dma_start(
                    wf_sb[:], wfT_d.ap().rearrange("(k p) o -> p k o", p=P))
                wm_sb = wlate.tile([P, FK, C], bf16)
                nc.scalar.dma_start(
                    wm_sb[:], wmT_d.ap().rearrange("(k p) o -> p k o", p=P))

                # ======== MLP in 4 token quarters ========
                for tq in range(4):
                    if tq == 2:
                        emit_x2_run(NR - 1)
                    t0 = tq * 512
                    hT = p_hT.tile([P, FK, 512], bf16, tag="hT")
                    for ft in range(FK):
                        ps = ppool.tile([P, 512], f32, tag="mm")
                        for ck in range(CK):
                            nc.tensor.matmul(
                                ps[:],
                                wf_sb[:, ck, ft * P : (ft + 1) * P],
                                xn2T[:, ck, t0 : t0 + 512],
                                start=(ck == 0), stop=(ck == CK - 1),
                            )
                        nc.scalar.activation(
                            out=hT[:, ft, :], in_=ps[:],
                            func=AF.Gelu_apprx_tanh,
                            bias=bfc_sb[:, ft : ft + 1], scale=1.0,
                        )
                    for tl in range(4):
                        out_sb = work.tile([P, C], f32, tag="f32buf")
                        for half in range(2):
                            ps = ppool.tile([P, 512], f32, tag="mm")
                            for fk in range(FK):
                                nc.tensor.matmul(
                                    ps[:],
                                    hT[:, fk, tl * P : (tl + 1) * P],
                                    wm_sb[:, fk, half * 512 : (half + 1) * 512],
                                    start=(fk == 0), stop=(fk == FK - 1),
                                )
                            nc.vector.tensor_copy(
                                out=out_sb[:, half * 512 : (half + 1) * 512],
                                in_=ps[:],
                            )
                        nc.gpsimd.dma_start(
                            out_mlp_d[t0 + tl * P : t0 + (tl + 1) * P, :],
                            out_sb[:],
                        )

            xn2T_cm.__exit__(None, None, None)

    nc.finalize()
    return nc


def _prep_inputs(x, w_attn, b_attn, w_proj, b_proj, w_fc, b_fc, w_mlp_proj):
    bf = ml_dtypes.bfloat16
    mask = np.triu(np.ones((P, P), dtype=np.float32)).astype(bf)
    in_maps = []
    for core in range(8):
        b, s = divmod(core, 2)
        wq = np.ascontiguousarray(w_attn[s * 512 : (s + 1) * 512, :].T).astype(bf)
        wk = np.ascontiguousarray(
            w_attn[C + s * 512 : C + (s + 1) * 512, :].T).astype(bf)
        wv = np.ascontiguousarray(
            w_attn[2 * C + s * 512 : 2 * C + (s + 1) * 512, :].T).astype(bf)
        bq = b_attn[s * 512 : (s + 1) * 512]
        bk = b_attn[C + s * 512 : C + (s + 1) * 512]
        bv = b_attn[2 * C + s * 512 : 2 * C + (s + 1) * 512]
        bqk = np.concatenate(
            [bq.reshape(EK, P).T, bk.reshape(EK, P).T], axis=1
        ).astype(np.float32)
        wp = np.ascontiguousarray(w_proj[:, s * 512 : (s + 1) * 512].T).astype(bf)
        wf = np.ascontiguousarray(w_fc[s * FPC : (s + 1) * FPC, :].T).astype(bf)
        bfc = np.ascontiguousarray(
            b_fc[s * FPC : (s + 1) * FPC].reshape(FK, P).T).astype(np.float32)
        wm = np.ascontiguousarray(
            w_mlp_proj[:, s * FPC : (s + 1) * FPC].T).astype(bf)
        in_maps.append(
            {
                "x": np.ascontiguousarray(x[b]),
                "wqT": wq, "wkT": wk, "wvT": wv, "wpT": wp, "wfT": wf, "wmT": wm,
                "bqk": bqk, "bv": np.ascontiguousarray(bv).astype(np.float32),
                "bproj": np.ascontiguousarray(b_proj).astype(np.float32),
                "bfc": bfc, "mask": mask,
            }
        )
    return in_maps


def run(x, w_attn, b_attn, w_proj, b_proj, w_fc, b_fc, w_mlp_proj, b_mlp_proj,
        trace=False):
    from concourse.bass_utils import run_bass_kernel_spmd

    if "nc" not in _CACHED:
        _CACHED["nc"] = _build_nc()
    nc = _CACHED["nc"]
    in_maps = _prep_inputs(
        x, w_attn, b_attn, w_proj, b_proj, w_fc, b_fc, w_mlp_proj
    )
    res = run_bass_kernel_spmd(
        nc, in_maps, core_ids=list(range(8)), trace=trace,
        trace_cores=list(range(8)) if trace else None,
    )
    out = np.empty((B, T, C), dtype=np.float32)
    for b in range(B):
        a = res.results[2 * b]
        c2 = res.results[2 * b + 1]
        out[b] = a["out_x2"] + a["out_mlp"] + c2["out_mlp"] + b_mlp_proj[None, :]
    return out, res


def kernel(x, w_attn, b_attn, w_proj, b_proj, w_fc, b_fc, w_mlp_proj, b_mlp_proj):
    out, _ = run(
        np.asarray(x, dtype=np.float32),
        np.asarray(w_attn, dtype=np.float32),
        np.asarray(b_attn, dtype=np.float32),
        np.asarray(w_proj, dtype=np.float32),
        np.asarray(b_proj, dtype=np.float32),
        np.asarray(w_fc, dtype=np.float32),
        np.asarray(b_fc, dtype=np.float32),
        np.asarray(w_mlp_proj, dtype=np.float32),
        np.asarray(b_mlp_proj, dtype=np.float32),
    )
    return out

